# revision 1
# baseline (speedup 1.0000x reference)
"""DGCNN (nn_DGCNN_type1) Trainium2 Bass kernel — self-contained.

Strategy: data-parallel over the 128 graphs, 16 per NeuronCore across 8 cores.
Per graph: fp32 kNN-score matmul (s = 2*f@f.T - |f_j|^2, row-constant dropped)
+ DVE max8/max_index/match_replace top-16; fp16 edge-MLP via dma_gather
(node-major fp16 tables, XBAR-transposed to feature-major) with fp32 PSUM
accumulation; k-max folded through the monotone LeakyReLU as a PSUM-side
reduce; lin1 + global-max-pool + MLP head on-chip.
"""

import numpy as np
import concourse.bacc as bacc
import concourse.mybir as mybir
from concourse.tile import TileContext
from concourse.masks import make_identity

F32, F16, I16, U16 = (mybir.dt.float32, mybir.dt.float16, mybir.dt.int16,
                      mybir.dt.uint16)
AF = mybir.ActivationFunctionType
ALU = mybir.AluOpType
AX = mybir.AxisListType

N = 512
K = 16
# timing-bisect knobs (default = full kernel)
GATHER_NIDX = None   # override num_idxs per gather (timing only; breaks data)
SKIP_TOPK = False
SKIP_LAYERS = False
NCHUNK = N // 128  # 4 row-chunks for the NxN score matrix


def host_prep(inputs, G, core):
    """Build the per-core in_map (numpy only: layout/dtype prep, no model math)."""
    f16 = np.float16
    x, pos, tq = inputs["x"], inputs["pos"], inputs["tq"]
    B_all = x.shape[0] // N
    xx = np.concatenate([tq, x, pos], axis=1).reshape(B_all, N, 5).astype(np.float32)
    sl = slice(core * G, (core + 1) * G)
    xxc = xx[sl]                                   # [G, 512, 5]
    feat5 = np.ascontiguousarray(xxc.transpose(0, 2, 1))         # [G, 5, 512] f32
    xtab1 = np.zeros((G, N, 128), f16)
    xtab1[:, :, 0:5] = xxc.astype(f16)

    w1a, w1b = inputs["w1a"], inputs["w1b"]
    w2a, w2b = inputs["w2a"], inputs["w2b"]
    w1botp = np.zeros((128, 64), f16)
    w1botp[0:5] = w1a[5:10].astype(f16)
    w2botp = np.zeros((128, 128), f16)
    w2botp[0:64] = w2a[64:128].astype(f16)
    wl1 = inputs["wl1"]

    return {
        "feat5": feat5,
        "feat5h": feat5.astype(f16),
        "xtab1": xtab1,
        "w1mod": np.ascontiguousarray((w1a[0:5] - w1a[5:10]).astype(f16)),
        "w1botp": w1botp,
        "w1b": np.ascontiguousarray(w1b.astype(f16)),
        "b1a": inputs["b1a"].reshape(64, 1).astype(np.float32),
        "b1b2": np.tile(inputs["b1b"], 2).reshape(128, 1).astype(np.float32),
        "w2mod": np.ascontiguousarray((w2a[0:64] - w2a[64:128]).astype(f16)),
        "w2botp": w2botp,
        "w2b": np.ascontiguousarray(w2b.astype(f16)),
        "b2a": inputs["b2a"].reshape(128, 1).astype(np.float32),
        "b2b2": np.tile(inputs["b2b"], 2).reshape(128, 1).astype(np.float32),
        "wl1xx": np.ascontiguousarray(wl1[0:5].astype(f16)),
        "wl1x1": np.ascontiguousarray(wl1[5:69].astype(f16)),
        "wl1x2": np.ascontiguousarray(wl1[69:133].astype(f16)),
        "bl1c": np.ascontiguousarray(inputs["bl1"].reshape(4, 128).T.astype(np.float32)),
        "wl2": np.ascontiguousarray(inputs["wl2"].astype(f16).reshape(4, 128, 256).transpose(1, 0, 2)),
        "bl2c": np.ascontiguousarray(inputs["bl2"].reshape(2, 128).T.astype(np.float32)),
        "wm1": np.ascontiguousarray(inputs["wm1"].astype(f16).reshape(2, 128, 128).transpose(1, 0, 2)),
        "bm1": inputs["bm1"].reshape(128, 1).astype(np.float32),
        "wm2": np.ascontiguousarray(inputs["wm2"].astype(f16)),
        "bm2": inputs["bm2"].reshape(3, 1).astype(np.float32),
    }


def declare_io(nc, G):
    t = {}
    def inp(name, shape, dt):
        t[name] = nc.dram_tensor(name, shape, dt, kind="ExternalInput")
    inp("feat5", [G, 5, N], F32)
    inp("feat5h", [G, 5, N], F16)
    inp("xtab1", [G, N, 128], F16)
    inp("w1mod", [5, 64], F16); inp("w1botp", [128, 64], F16)
    inp("w1b", [64, 64], F16); inp("b1a", [64, 1], F32); inp("b1b2", [128, 1], F32)
    inp("w2mod", [64, 128], F16); inp("w2botp", [128, 128], F16)
    inp("w2b", [128, 64], F16); inp("b2a", [128, 1], F32); inp("b2b2", [128, 1], F32)
    inp("wl1xx", [5, N], F16); inp("wl1x1", [64, N], F16); inp("wl1x2", [64, N], F16)
    inp("bl1c", [128, 4], F32); inp("wl2", [128, 4, 256], F16); inp("bl2c", [128, 2], F32)
    inp("wm1", [128, 2, 128], F16); inp("bm1", [128, 1], F32)
    inp("wm2", [128, 3], F16); inp("bm2", [3, 1], F32)
    t["o"] = nc.dram_tensor("o", [3, G], F32, kind="ExternalOutput")
    return t


def build(nc, G, reps=1):
    t = declare_io(nc, G)
    with TileContext(nc) as tc:
        _build_body(nc, tc, t, G, reps)
    nc.compile()
    return t


def _build_body(nc, tc, t, G, reps=1):
    sbw = tc.alloc_tile_pool(name="sbw", bufs=1)          # persistent
    sb = tc.alloc_tile_pool(name="sb", bufs=2)            # rotating tiles
    sb3 = tc.alloc_tile_pool(name="sb3", bufs=3)          # deeper rotation
    ps_nd = tc.alloc_tile_pool(name="ps_nd", bufs=2, space="PSUM")   # 2 banks
    ps_a1 = tc.alloc_tile_pool(name="ps_a1", bufs=2, space="PSUM")   # 2 banks
    ps_a2 = tc.alloc_tile_pool(name="ps_a2", bufs=1, space="PSUM")   # 2 banks
    ps_sm = tc.alloc_tile_pool(name="ps_sm", bufs=1, space="PSUM")   # 1 bank
    dram = tc.alloc_tile_pool(name="dram", bufs=1, space="DRAM")

    # ---- persistent weight tiles ----
    w = {}
    for name in ["w1mod", "w1botp", "w1b", "w2mod", "w2botp", "w2b",
                 "wl1xx", "wl1x1", "wl1x2", "wl2", "wm1", "wm2"]:
        w[name] = sbw.tile(list(t[name].shape), F16, tag=name, name='w_'+name)
        nc.sync.dma_start(out=w[name][:], in_=t[name][:])
    for name in ["b1a", "b1b2", "b2a", "b2b2", "bl1c", "bl2c", "bm1", "bm2"]:
        w[name] = sbw.tile(list(t[name].shape), F32, tag=name, name='b_'+name)
        nc.sync.dma_start(out=w[name][:], in_=t[name][:])
    ident = sbw.tile([128, 128], F16, tag="ident")
    make_identity(nc, ident[:])
    ones = sbw.tile([64, 1], F32, tag="ones")
    nc.gpsimd.memset(ones[:], 1.0)
    onesrow = sbw.tile([1, N], F32, tag="onesrow")
    nc.gpsimd.memset(onesrow[:], 1.0)

    # persistent gather-index tiles (rows 16-127 must hold valid values)
    NIDX_SLOTS = 3
    idx_tiles = []
    for s in range(NIDX_SLOTS):
        it = sbw.tile([128, N], I16, tag=f"idxs{s}", name=f"idxs{s}")
        nc.gpsimd.memset(it[:], 0)
        idx_tiles.append(it)
    idx_slot = [0]

    x1tab = dram.tile([G, N, 128], F16, tag="x1tab")

    Gt_lo = sbw.tile([128, G], F32, tag="gtlo")
    Gt_hi = sbw.tile([128, G], F32, tag="gthi")

    rep_ctx = tc.For_i(0, reps, 1) if reps > 1 else None
    if rep_ctx is not None:
        rep_ctx.__enter__()

    def conv(d, Fmid, wmod, wbotp, wsec, ba, bb2, gtab_ap, fill_B, featTh,
             kact_sink):
        """One DynamicEdgeConv. kact_sink(tt, kact): kact [128, 64] f32,
        rows 64h+f = feature f of node group h; tile tt covers nodes
        [128*tt + 64*h, +64)."""
        dp = d + 1
        B = sb.tile([65, N], F32, tag="Btile")
        fill_B(B)                              # fills B[0:d, :] (f32)
        sc = sb.tile([65, N], F32, tag="sctile")
        nc.vector.tensor_scalar_mul(sc[0:d, :], B[0:d, :], 2.0)
        F2 = sb.tile([64, N], F32, tag="F2tile")
        nc.scalar.activation(F2[0:d, :], B[0:d, :], AF.Square)
        sqp = ps_sm.tile([1, N], F32, tag="sm")
        nc.tensor.matmul(out=sqp[:], lhsT=ones[0:d, :], rhs=F2[0:d, :],
                         start=True, stop=True)
        if d % 32 == 0:
            nc.gpsimd.memset(sc[d:d + 1, :], 1.0)
            nc.scalar.activation(B[d:d + 1, :], sqp[:], AF.Identity, scale=-1.0)
        else:
            nc.sync.dma_start(out=sc[d:d + 1, :], in_=onesrow[:])
            sqtmp = sb.tile([1, N], F32, tag="sqtmp")
            nc.scalar.activation(sqtmp[:], sqp[:], AF.Identity, scale=-1.0)
            nc.sync.dma_start(out=B[d:d + 1, :], in_=sqtmp[:])

        # ---- negd2 chunks + topk -> wrapped idx tile ----
        idxs = idx_tiles[idx_slot[0] % NIDX_SLOTS]
        idx_slot[0] += 1
        idxTp = ps_sm.tile([16, N], F16, tag="sm")
        for c in range(NCHUNK):
            nd_p = ps_nd.tile([128, N], F32, tag="ndp")
            nc.tensor.matmul(out=nd_p[:], lhsT=sc[0:dp, 128 * c:128 * (c + 1)],
                             rhs=B[0:dp, :], start=True, stop=True)
            nd = sb.tile([128, N], F32, tag="ndsb")
            nc.scalar.activation(nd[:], nd_p[:], AF.Copy)
            maxv = sb.tile([128, 16], F32, tag="maxv")
            maxi = sb.tile([128, 16], U16, tag="maxi")
            if not SKIP_TOPK:
                nc.vector.max(out=maxv[:, 0:8], in_=nd[:])
                nc.vector.max_index(out=maxi[:, 0:8], in_max=maxv[:, 0:8], in_values=nd[:])
                nc.vector.match_replace(out=nd[:], in_to_replace=maxv[:, 0:8],
                                        in_values=nd[:], imm_value=-1e30)
                nc.vector.max(out=maxv[:, 8:16], in_=nd[:])
                nc.vector.max_index(out=maxi[:, 8:16], in_max=maxv[:, 8:16], in_values=nd[:])
            else:
                nc.vector.memset(maxi[:], 0)
            mif = sb.tile([128, 16], F16, tag="mif")
            nc.vector.tensor_copy(mif[:], maxi[:])
            nc.tensor.transpose(out=idxTp[:, 128 * c:128 * (c + 1)], in_=mif[:],
                                identity=ident[:])
        # rows 0-15: full wrapped idxT (read by the sim + rx cpu);
        # rows 16-31: copy for the tx cpu (reads partitions 16-31).
        nc.vector.tensor_copy(idxs[0:16, :], idxTp[:])
        nc.sync.dma_start(out=idxs[16:32, :], in_=idxs[0:16, :])

        # ---- gather xj: node-major table -> feature-major [128, 8192] f16 ----
        xjg = sb.tile([128, K * N], F16, tag="xjg")
        ni = GATHER_NIDX or (K * N)
        nc.gpsimd.dma_gather(out_ap=xjg[:, None, 0:ni], in_ap=gtab_ap,
                             idxs_ap=idxs[:, 0:max(ni // 16, 32)], num_idxs=ni,
                             num_idxs_reg=ni, elem_size=128, transpose=True,
                             single_packet=False)
        if ni < K * N:
            nc.gpsimd.memset(xjg[:, ni:], 0)

        # ---- layer1 + layer2 + kmax ----
        for tt in range(4):
            a2 = ps_a2.tile([128, 1024], F32, tag="a2")
            for q in range(4):
                h, r = q // 2, q % 2
                c = 4 * tt + 2 * h + r
                a1 = ps_a1.tile([128, N], F32, tag="a1")
                nc.tensor.matmul(out=a1[0:Fmid, :], lhsT=wmod[:],
                                 rhs=featTh[:, 32 * c:32 * (c + 1), None]
                                 .to_broadcast([d, 32, K]),
                                 start=True, stop=False)
                nc.tensor.matmul(out=a1[0:Fmid, :], lhsT=wbotp[:],
                                 rhs=xjg[:, N * c:N * (c + 1)],
                                 start=False, stop=True)
                h1 = sb3.tile([128, N], F16, tag="h1")
                nc.scalar.activation(h1[0:Fmid, :], a1[0:Fmid, :], AF.Prelu,
                                     bias=ba[:], alpha=0.01)
                nc.tensor.matmul(out=a2[64 * h:64 * h + 64, 512 * r:512 * (r + 1)],
                                 lhsT=wsec[:], rhs=h1[0:Fmid, :],
                                 start=True, stop=True)
            kmx = sb.tile([128, 64], F32, tag="kmx")
            nc.vector.tensor_reduce(out=kmx[:], in_=a2[:].rearrange(
                "p (n k) -> p n k", k=K), op=ALU.max, axis=AX.X)
            kact = sb.tile([128, 64], F32, tag="kact")
            nc.scalar.activation(kact[:], kmx[:], AF.Prelu, bias=bb2[:], alpha=0.01)
            kact_sink(tt, kact)

    for g in range(G):
        # ===================== conv1 =====================
        def fill_B_conv1(B, g=g):
            nc.sync.dma_start(out=B[0:5, :], in_=t["feat5"][g])
        f5h = sb.tile([5, N], F16, tag="f5h")
        nc.sync.dma_start(out=f5h[:], in_=t["feat5h"][g])
        x1f32 = sb.tile([64, N], F32, tag="x1f32")
        x1f16 = sb.tile([64, N], F16, tag="x1f16")

        def sink1(tt, kact):
            for h in range(2):
                cols = slice(128 * tt + 64 * h, 128 * tt + 64 * h + 64)
                nc.vector.tensor_copy(x1f32[:, cols], kact[64 * h:64 * h + 64, :])
                nc.scalar.activation(x1f16[:, cols], kact[64 * h:64 * h + 64, :],
                                     AF.Copy)

        conv(5, 64, w["w1mod"], w["w1botp"], w["w1b"], w["b1a"], w["b1b2"],
             t["xtab1"][g], fill_B_conv1, f5h, sink1)

        # x1 node-major staging -> DRAM table (cols 0:64 = x1, 64:128 = dup)
        x1nmp = ps_sm.tile([128, 256], F16, tag="sm")
        for c in range(NCHUNK):
            nc.tensor.transpose(out=x1nmp[:, 64 * c:64 * (c + 1)],
                                in_=x1f16[:, 128 * c:128 * (c + 1)],
                                identity=ident[0:64, 0:64])
        x1nm = sb.tile([128, 4, 128], F16, tag="x1nm")
        nc.scalar.activation(x1nm[:, :, 0:64],
                             x1nmp[:].rearrange("p (a b) -> p a b", b=64), AF.Copy)
        nc.scalar.activation(x1nm[:, :, 64:128],
                             x1nmp[:].rearrange("p (a b) -> p a b", b=64), AF.Copy)
        nc.sync.dma_start(out=x1tab[g].rearrange("(c p) f -> p c f", p=128),
                          in_=x1nm[:])

        # ===================== conv2 =====================
        def fill_B_conv2(B):
            nc.vector.tensor_copy(B[0:64, :], x1f32[:])
        x2f16 = sb.tile([64, N], F16, tag="x2f16")

        def sink2(tt, kact):
            for h in range(2):
                cols = slice(128 * tt + 64 * h, 128 * tt + 64 * h + 64)
                nc.scalar.activation(x2f16[:, cols], kact[64 * h:64 * h + 64, :],
                                     AF.Copy)

        conv(64, 128, w["w2mod"], w["w2botp"], w["w2b"], w["b2a"], w["b2b2"],
             x1tab[g], fill_B_conv2, x1f16, sink2)

        # ===================== lin1 + pool =====================
        for fo in range(2):
            h2p = ps_a2.tile([128, N], F32, tag="a2")
            for c in range(NCHUNK):
                hp = ps_a1.tile([128, N], F32, tag="a1")
                nc.tensor.matmul(out=hp[:], lhsT=w["wl1xx"][:, 128 * c:128 * (c + 1)],
                                 rhs=f5h[:], start=True, stop=False)
                nc.tensor.matmul(out=hp[:], lhsT=w["wl1x1"][:, 128 * c:128 * (c + 1)],
                                 rhs=x1f16[:], start=False, stop=False)
                nc.tensor.matmul(out=hp[:], lhsT=w["wl1x2"][:, 128 * c:128 * (c + 1)],
                                 rhs=x2f16[:], start=False, stop=True)
                hsb = sb3.tile([128, N], F16, tag="h1")
                nc.scalar.activation(hsb[:], hp[:], AF.Prelu,
                                     bias=w["bl1c"][:, c:c + 1], alpha=0.01)
                nc.tensor.matmul(out=h2p[:],
                                 lhsT=w["wl2"][:, c, 128 * fo:128 * (fo + 1)],
                                 rhs=hsb[:], start=(c == 0), stop=(c == NCHUNK - 1))
            gt = Gt_lo if fo == 0 else Gt_hi
            nc.vector.tensor_reduce(out=gt[:, g:g + 1], in_=h2p[:], op=ALU.max,
                                    axis=AX.X)

    # ===================== head =====================
    t1p = ps_sm.tile([128, G], F32, tag="sm")
    for fo in range(2):
        gt = Gt_lo if fo == 0 else Gt_hi
        ga = sb.tile([128, G], F16, tag="ga")
        nc.scalar.activation(ga[:], gt[:], AF.Prelu, bias=w["bl2c"][:, fo:fo + 1],
                             alpha=0.01)
        nc.tensor.matmul(out=t1p[:], lhsT=w["wm1"][:, fo, :],
                         rhs=ga[:], start=(fo == 0), stop=(fo == 1))
    t1 = sb.tile([128, G], F16, tag="t1")
    nc.scalar.activation(t1[:], t1p[:], AF.Prelu, bias=w["bm1"][:], alpha=0.01)
    outp = ps_sm.tile([3, G], F32, tag="sm")
    nc.tensor.matmul(out=outp[:], lhsT=w["wm2"][:], rhs=t1[:], start=True, stop=True)
    outsb = sb.tile([3, G], F32, tag="outsb")
    nc.scalar.activation(outsb[:], outp[:], AF.Identity, bias=w["bm2"][:])
    nc.sync.dma_start(out=t["o"][:], in_=outsb[:])

    if rep_ctx is not None:
        rep_ctx.__exit__(None, None, None)

    for pool in (dram, ps_sm, ps_a2, ps_a1, ps_nd, sb3, sb, sbw):
        pool.release()


# ======================= harness entry point =======================
_CACHE = {}


def _get_program(G):
    if "nc" not in _CACHE:
        import concourse.bacc as _bacc
        nc = _bacc.Bacc()
        build(nc, G)
        _CACHE["nc"] = nc
    return _CACHE["nc"]


def kernel(x, pos, tq, batch, w1a, b1a, w1b, b1b, w2a, b2a, w2b, b2b,
           wl1, bl1, wl2, bl2, wm1, bm1, wm2, bm2):
    """Full-input entry: shards graphs over 8 NeuronCores, returns [128, 3]."""
    from concourse.bass_utils import run_bass_kernel_spmd
    inputs = dict(x=np.asarray(x), pos=np.asarray(pos), tq=np.asarray(tq),
                  w1a=np.asarray(w1a), b1a=np.asarray(b1a),
                  w1b=np.asarray(w1b), b1b=np.asarray(b1b),
                  w2a=np.asarray(w2a), b2a=np.asarray(b2a),
                  w2b=np.asarray(w2b), b2b=np.asarray(b2b),
                  wl1=np.asarray(wl1), bl1=np.asarray(bl1),
                  wl2=np.asarray(wl2), bl2=np.asarray(bl2),
                  wm1=np.asarray(wm1), bm1=np.asarray(bm1),
                  wm2=np.asarray(wm2), bm2=np.asarray(bm2))
    NCORES = 8
    B_all = inputs["x"].shape[0] // N
    G = B_all // NCORES
    nc = _get_program(G)
    in_maps = [host_prep(inputs, G, c) for c in range(NCORES)]
    res = run_bass_kernel_spmd(nc, in_maps, core_ids=list(range(NCORES)))
    out = np.concatenate([res.results[c]["o"].T for c in range(NCORES)], axis=0)
    return out.astype(np.float32)



# revision 10
# speedup vs baseline: 1.0169x; 1.0169x over previous
"""DGCNN (nn_DGCNN_type1) Trainium2 Bass kernel — self-contained.

Strategy: data-parallel over the 128 graphs, 16 per NeuronCore across 8 cores.
Per graph: fp32 kNN-score matmul (s = 2*f@f.T - |f_j|^2, row-constant dropped)
+ DVE max8/max_index/match_replace top-16; fp16 edge-MLP via dma_gather
(node-major fp16 tables, XBAR-transposed to feature-major) with fp32 PSUM
accumulation; k-max folded through the monotone LeakyReLU as a PSUM-side
reduce; lin1 + global-max-pool + MLP head on-chip.
"""

import numpy as np
import concourse.bacc as bacc
import concourse.mybir as mybir
from concourse.tile import TileContext
from concourse.masks import make_identity

F32, F16, I16, U16 = (mybir.dt.float32, mybir.dt.float16, mybir.dt.int16,
                      mybir.dt.uint16)
AF = mybir.ActivationFunctionType
ALU = mybir.AluOpType
AX = mybir.AxisListType

N = 512
K = 16
# timing-bisect knobs (default = full kernel)
GATHER_NIDX = None   # override num_idxs per gather (timing only; breaks data)
SKIP_TOPK = False
SKIP_LAYERS = False
NCHUNK = N // 128  # 4 row-chunks for the NxN score matrix


def host_prep(inputs, G, core):
    """Build the per-core in_map (numpy only: layout/dtype prep, no model math)."""
    f16 = np.float16
    x, pos, tq = inputs["x"], inputs["pos"], inputs["tq"]
    B_all = x.shape[0] // N
    xx = np.concatenate([tq, x, pos], axis=1).reshape(B_all, N, 5).astype(np.float32)
    sl = slice(core * G, (core + 1) * G)
    xxc = xx[sl]                                   # [G, 512, 5]
    feat5 = np.ascontiguousarray(xxc.transpose(0, 2, 1))         # [G, 5, 512] f32
    # SBUF-gather layout: token n at partition n%128, rank n//128 (256B each);
    # partition-major so the one-shot preload DMA is contiguous
    xtab1 = np.zeros((128, G, NCHUNK, 128), f16)
    xtab1[:, :, :, 0:5] = xxc.reshape(G, NCHUNK, 128, 5).transpose(2, 0, 1, 3).astype(f16)

    w1a, w1b = inputs["w1a"], inputs["w1b"]
    w2a, w2b = inputs["w2a"], inputs["w2b"]
    w1botp = np.zeros((128, 64), f16)
    w1botp[0:5] = w1a[5:10].astype(f16)
    w2botp = np.zeros((128, 128), f16)
    w2botp[0:64] = w2a[64:128].astype(f16)
    wl1 = inputs["wl1"]

    return {
        "feat5": feat5,
        "feat5h": feat5.astype(f16),
        "xtab1": xtab1,
        "w1mod": np.ascontiguousarray((w1a[0:5] - w1a[5:10]).astype(f16)),
        "w1botp": w1botp,
        "w1b": np.ascontiguousarray(w1b.astype(f16)),
        "b1a": inputs["b1a"].reshape(64, 1).astype(np.float32),
        "b1b2": np.tile(inputs["b1b"], 2).reshape(128, 1).astype(np.float32),
        "w2mod": np.ascontiguousarray((w2a[0:64] - w2a[64:128]).astype(f16)),
        "w2botp": w2botp,
        "w2b": np.ascontiguousarray(w2b.astype(f16)),
        "b2a": inputs["b2a"].reshape(128, 1).astype(np.float32),
        "b2b2": np.tile(inputs["b2b"], 2).reshape(128, 1).astype(np.float32),
        "wl1xx": np.ascontiguousarray(wl1[0:5].astype(f16)),
        "wl1x1": np.ascontiguousarray(wl1[5:69].astype(f16)),
        "wl1x2": np.ascontiguousarray(wl1[69:133].astype(f16)),
        "bl1c": np.ascontiguousarray(inputs["bl1"].reshape(4, 128).T.astype(np.float32)),
        "wl2": np.ascontiguousarray(inputs["wl2"].astype(f16).reshape(4, 128, 256).transpose(1, 0, 2)),
        "bl2c": np.ascontiguousarray(inputs["bl2"].reshape(2, 128).T.astype(np.float32)),
        "wm1": np.ascontiguousarray(inputs["wm1"].astype(f16).reshape(2, 128, 128).transpose(1, 0, 2)),
        "bm1": inputs["bm1"].reshape(128, 1).astype(np.float32),
        "wm2": np.ascontiguousarray(inputs["wm2"].astype(f16)),
        "bm2": inputs["bm2"].reshape(3, 1).astype(np.float32),
    }


def declare_io(nc, G):
    t = {}
    def inp(name, shape, dt):
        t[name] = nc.dram_tensor(name, shape, dt, kind="ExternalInput")
    inp("feat5", [G, 5, N], F32)
    inp("feat5h", [G, 5, N], F16)
    inp("xtab1", [128, G, NCHUNK, 128], F16)
    inp("w1mod", [5, 64], F16); inp("w1botp", [128, 64], F16)
    inp("w1b", [64, 64], F16); inp("b1a", [64, 1], F32); inp("b1b2", [128, 1], F32)
    inp("w2mod", [64, 128], F16); inp("w2botp", [128, 128], F16)
    inp("w2b", [128, 64], F16); inp("b2a", [128, 1], F32); inp("b2b2", [128, 1], F32)
    inp("wl1xx", [5, N], F16); inp("wl1x1", [64, N], F16); inp("wl1x2", [64, N], F16)
    inp("bl1c", [128, 4], F32); inp("wl2", [128, 4, 256], F16); inp("bl2c", [128, 2], F32)
    inp("wm1", [128, 2, 128], F16); inp("bm1", [128, 1], F32)
    inp("wm2", [128, 3], F16); inp("bm2", [3, 1], F32)
    t["o"] = nc.dram_tensor("o", [3, G], F32, kind="ExternalOutput")
    return t


def build(nc, G, reps=1):
    t = declare_io(nc, G)
    with TileContext(nc) as tc:
        _build_body(nc, tc, t, G, reps)
    nc.compile()
    return t


def _build_body(nc, tc, t, G, reps=1):
    sbw = tc.alloc_tile_pool(name="sbw", bufs=1)          # persistent
    sb = tc.alloc_tile_pool(name="sb", bufs=2)            # rotating tiles
    sb3 = tc.alloc_tile_pool(name="sb3", bufs=3)          # deeper rotation
    ps_nd = tc.alloc_tile_pool(name="ps_nd", bufs=2, space="PSUM")   # 2 banks
    ps_a1 = tc.alloc_tile_pool(name="ps_a1", bufs=2, space="PSUM")   # 2 banks
    ps_a2 = tc.alloc_tile_pool(name="ps_a2", bufs=1, space="PSUM")   # 2 banks
    ps_sm = tc.alloc_tile_pool(name="ps_sm", bufs=1, space="PSUM")   # 1 bank

    # ---- persistent weight tiles ----
    w = {}
    for name in ["w1mod", "w1botp", "w1b", "w2mod", "w2botp", "w2b",
                 "wl1xx", "wl1x1", "wl1x2", "wl2", "wm1", "wm2"]:
        w[name] = sbw.tile(list(t[name].shape), F16, tag=name, name='w_'+name)
        nc.sync.dma_start(out=w[name][:], in_=t[name][:])
    for name in ["b1a", "b1b2", "b2a", "b2b2", "bl1c", "bl2c", "bm1", "bm2"]:
        w[name] = sbw.tile(list(t[name].shape), F32, tag=name, name='b_'+name)
        nc.sync.dma_start(out=w[name][:], in_=t[name][:])
    ident = sbw.tile([128, 128], F16, tag="ident")
    make_identity(nc, ident[:])
    ones = sbw.tile([64, 1], F32, tag="ones")
    nc.gpsimd.memset(ones[:], 1.0)
    onesrow = sbw.tile([1, N], F32, tag="onesrow")
    nc.gpsimd.memset(onesrow[:], 1.0)

    # persistent gather-index tiles (rows 16-127 must hold valid values)
    NIDX_SLOTS = 3
    idx_tiles = []
    for s in range(NIDX_SLOTS):
        it = sbw.tile([128, N], I16, tag=f"idxs{s}", name=f"idxs{s}")
        nc.gpsimd.memset(it[:], 0)
        idx_tiles.append(it)
    idx_slot = [0]

    # conv1 gather table for all G graphs, resident in SBUF (16 KiB/partition)
    xtab_sb = sbw.tile([128, G, NCHUNK, 128], F16, tag="xtab_sb")
    nc.sync.dma_start(out=xtab_sb[:], in_=t["xtab1"][:])

    Gt_lo = sbw.tile([128, G], F32, tag="gtlo")
    Gt_hi = sbw.tile([128, G], F32, tag="gthi")

    rep_ctx = tc.For_i(0, reps, 1) if reps > 1 else None
    if rep_ctx is not None:
        rep_ctx.__enter__()

    def conv(d, Fmid, wmod, wbotp, wsec, ba, bb2, gtab_ap, fill_B, featTh,
             kact_sink):
        """One DynamicEdgeConv. kact_sink(tt, kact): kact [128, 64] f32,
        rows 64h+f = feature f of node group h; tile tt covers nodes
        [128*tt + 64*h, +64)."""
        dp = d + 1
        B = sb.tile([65, N], F32, tag="Btile")
        fill_B(B)                              # fills B[0:d, :] (f32)
        sc = sb.tile([65, N], F32, tag="sctile")
        nc.vector.tensor_scalar_mul(sc[0:d, :], B[0:d, :], 2.0)
        F2 = sb.tile([64, N], F32, tag="F2tile")
        nc.scalar.activation(F2[0:d, :], B[0:d, :], AF.Square)
        sqp = ps_sm.tile([1, N], F32, tag="sm")
        nc.tensor.matmul(out=sqp[:], lhsT=ones[0:d, :], rhs=F2[0:d, :],
                         start=True, stop=True)
        if d % 32 == 0:
            nc.gpsimd.memset(sc[d:d + 1, :], 1.0)
            nc.scalar.activation(B[d:d + 1, :], sqp[:], AF.Identity, scale=-1.0)
        else:
            nc.sync.dma_start(out=sc[d:d + 1, :], in_=onesrow[:])
            sqtmp = sb.tile([1, N], F32, tag="sqtmp")
            nc.scalar.activation(sqtmp[:], sqp[:], AF.Identity, scale=-1.0)
            nc.sync.dma_start(out=B[d:d + 1, :], in_=sqtmp[:])

        # ---- negd2 chunks + topk -> wrapped idx tile ----
        idxs = idx_tiles[idx_slot[0] % NIDX_SLOTS]
        idx_slot[0] += 1
        idxTp = ps_sm.tile([16, N], F16, tag="sm")
        for c in range(NCHUNK):
            nd_p = ps_nd.tile([128, N], F32, tag="ndp")
            nc.tensor.matmul(out=nd_p[:], lhsT=sc[0:dp, 128 * c:128 * (c + 1)],
                             rhs=B[0:dp, :], start=True, stop=True)
            nd = sb.tile([128, N], F32, tag="ndsb")
            nc.scalar.activation(nd[:], nd_p[:], AF.Copy)
            maxv = sb.tile([128, 16], F32, tag="maxv")
            maxi = sb.tile([128, 16], U16, tag="maxi")
            if not SKIP_TOPK:
                nc.vector.max(out=maxv[:, 0:8], in_=nd[:])
                nc.vector.max_index(out=maxi[:, 0:8], in_max=maxv[:, 0:8], in_values=nd[:])
                nc.vector.match_replace(out=nd[:], in_to_replace=maxv[:, 0:8],
                                        in_values=nd[:], imm_value=-1e30)
                nc.vector.max(out=maxv[:, 8:16], in_=nd[:])
                nc.vector.max_index(out=maxi[:, 8:16], in_max=maxv[:, 8:16], in_values=nd[:])
            else:
                nc.vector.memset(maxi[:], 0)
            mif = sb.tile([128, 16], F16, tag="mif")
            nc.vector.tensor_copy(mif[:], maxi[:])
            nc.tensor.transpose(out=idxTp[:, 128 * c:128 * (c + 1)], in_=mif[:],
                                identity=ident[:])
        # rows 0-15: full wrapped idxT (read by the sim + rx cpu);
        # rows 16-31: copy for the tx cpu (reads partitions 16-31).
        nc.vector.tensor_copy(idxs[0:16, :], idxTp[:])
        nc.sync.dma_start(out=idxs[16:32, :], in_=idxs[0:16, :])

        # ---- gather xj from SBUF table -> feature-major [128, 8192] f16 ----
        xjg = sb.tile([128, K * N], F16, tag="xjg")
        ni = GATHER_NIDX or (K * N)
        nc.gpsimd.dma_gather(out_ap=xjg[:, None, 0:ni], in_ap=gtab_ap,
                             idxs_ap=idxs[:, 0:max(ni // 16, 32)], num_idxs=ni,
                             num_idxs_reg=ni, elem_size=128, transpose=True,
                             single_packet=False, sbuf_tokens_per_rank=128,
                             sbuf_free_dim_per_rank=256)
        if ni < K * N:
            nc.gpsimd.memset(xjg[:, ni:], 0)

        # ---- layer1 + layer2 + kmax ----
        for tt in range(4):
            a2 = ps_a2.tile([128, 1024], F32, tag="a2")
            for q in range(4):
                h, r = q // 2, q % 2
                c = 4 * tt + 2 * h + r
                a1 = ps_a1.tile([128, N], F32, tag="a1")
                nc.tensor.matmul(out=a1[0:Fmid, :], lhsT=wmod[:],
                                 rhs=featTh[:, 32 * c:32 * (c + 1), None]
                                 .to_broadcast([d, 32, K]),
                                 start=True, stop=False)
                nc.tensor.matmul(out=a1[0:Fmid, :], lhsT=wbotp[:],
                                 rhs=xjg[:, N * c:N * (c + 1)],
                                 start=False, stop=True)
                h1 = sb3.tile([128, N], F16, tag="h1")
                nc.scalar.activation(h1[0:Fmid, :], a1[0:Fmid, :], AF.Prelu,
                                     bias=ba[:], alpha=0.01)
                nc.tensor.matmul(out=a2[64 * h:64 * h + 64, 512 * r:512 * (r + 1)],
                                 lhsT=wsec[:], rhs=h1[0:Fmid, :],
                                 start=True, stop=True)
            kmx = sb.tile([128, 64], F32, tag="kmx")
            nc.vector.tensor_reduce(out=kmx[:], in_=a2[:].rearrange(
                "p (n k) -> p n k", k=K), op=ALU.max, axis=AX.X)
            kact = sb.tile([128, 64], F32, tag="kact")
            nc.scalar.activation(kact[:], kmx[:], AF.Prelu, bias=bb2[:], alpha=0.01)
            kact_sink(tt, kact)

    for g in range(G):
        # ===================== conv1 =====================
        def fill_B_conv1(B, g=g):
            nc.sync.dma_start(out=B[0:5, :], in_=t["feat5"][g])
        f5h = sb.tile([5, N], F16, tag="f5h")
        nc.sync.dma_start(out=f5h[:], in_=t["feat5h"][g])
        x1f32 = sb.tile([64, N], F32, tag="x1f32")
        x1f16 = sb.tile([64, N], F16, tag="x1f16")

        def sink1(tt, kact):
            for h in range(2):
                cols = slice(128 * tt + 64 * h, 128 * tt + 64 * h + 64)
                nc.vector.tensor_copy(x1f32[:, cols], kact[64 * h:64 * h + 64, :])
                nc.scalar.activation(x1f16[:, cols], kact[64 * h:64 * h + 64, :],
                                     AF.Copy)

        conv(5, 64, w["w1mod"], w["w1botp"], w["w1b"], w["b1a"], w["b1b2"],
             xtab_sb[:, g], fill_B_conv1, f5h, sink1)

        # x1 node-major staging -> SBUF table (cols 0:64 = x1, 64:128 = dup)
        x1nmp = ps_sm.tile([128, 256], F16, tag="sm")
        for c in range(NCHUNK):
            nc.tensor.transpose(out=x1nmp[:, 64 * c:64 * (c + 1)],
                                in_=x1f16[:, 128 * c:128 * (c + 1)],
                                identity=ident[0:64, 0:64])
        x1nm = sb.tile([128, 4, 128], F16, tag="x1nm")
        nc.scalar.activation(x1nm[:, :, 0:64],
                             x1nmp[:].rearrange("p (a b) -> p a b", b=64), AF.Copy)
        nc.scalar.activation(x1nm[:, :, 64:128],
                             x1nmp[:].rearrange("p (a b) -> p a b", b=64), AF.Copy)

        # ===================== conv2 =====================
        def fill_B_conv2(B):
            nc.vector.tensor_copy(B[0:64, :], x1f32[:])
        x2f16 = sb.tile([64, N], F16, tag="x2f16")

        def sink2(tt, kact):
            for h in range(2):
                cols = slice(128 * tt + 64 * h, 128 * tt + 64 * h + 64)
                nc.scalar.activation(x2f16[:, cols], kact[64 * h:64 * h + 64, :],
                                     AF.Copy)

        conv(64, 128, w["w2mod"], w["w2botp"], w["w2b"], w["b2a"], w["b2b2"],
             x1nm[:], fill_B_conv2, x1f16, sink2)

        # ===================== lin1 + pool =====================
        for fo in range(2):
            h2p = ps_a2.tile([128, N], F32, tag="a2")
            for c in range(NCHUNK):
                hp = ps_a1.tile([128, N], F32, tag="a1")
                nc.tensor.matmul(out=hp[:], lhsT=w["wl1xx"][:, 128 * c:128 * (c + 1)],
                                 rhs=f5h[:], start=True, stop=False)
                nc.tensor.matmul(out=hp[:], lhsT=w["wl1x1"][:, 128 * c:128 * (c + 1)],
                                 rhs=x1f16[:], start=False, stop=False)
                nc.tensor.matmul(out=hp[:], lhsT=w["wl1x2"][:, 128 * c:128 * (c + 1)],
                                 rhs=x2f16[:], start=False, stop=True)
                hsb = sb3.tile([128, N], F16, tag="h1")
                nc.scalar.activation(hsb[:], hp[:], AF.Prelu,
                                     bias=w["bl1c"][:, c:c + 1], alpha=0.01)
                nc.tensor.matmul(out=h2p[:],
                                 lhsT=w["wl2"][:, c, 128 * fo:128 * (fo + 1)],
                                 rhs=hsb[:], start=(c == 0), stop=(c == NCHUNK - 1))
            gt = Gt_lo if fo == 0 else Gt_hi
            nc.vector.tensor_reduce(out=gt[:, g:g + 1], in_=h2p[:], op=ALU.max,
                                    axis=AX.X)

    # ===================== head =====================
    t1p = ps_sm.tile([128, G], F32, tag="sm")
    for fo in range(2):
        gt = Gt_lo if fo == 0 else Gt_hi
        ga = sb.tile([128, G], F16, tag="ga")
        nc.scalar.activation(ga[:], gt[:], AF.Prelu, bias=w["bl2c"][:, fo:fo + 1],
                             alpha=0.01)
        nc.tensor.matmul(out=t1p[:], lhsT=w["wm1"][:, fo, :],
                         rhs=ga[:], start=(fo == 0), stop=(fo == 1))
    t1 = sb.tile([128, G], F16, tag="t1")
    nc.scalar.activation(t1[:], t1p[:], AF.Prelu, bias=w["bm1"][:], alpha=0.01)
    outp = ps_sm.tile([3, G], F32, tag="sm")
    nc.tensor.matmul(out=outp[:], lhsT=w["wm2"][:], rhs=t1[:], start=True, stop=True)
    outsb = sb.tile([3, G], F32, tag="outsb")
    nc.scalar.activation(outsb[:], outp[:], AF.Identity, bias=w["bm2"][:])
    nc.sync.dma_start(out=t["o"][:], in_=outsb[:])

    if rep_ctx is not None:
        rep_ctx.__exit__(None, None, None)

    for pool in (ps_sm, ps_a2, ps_a1, ps_nd, sb3, sb, sbw):
        pool.release()


# ======================= harness entry point =======================
_CACHE = {}


def _get_program(G):
    if "nc" not in _CACHE:
        import concourse.bacc as _bacc
        nc = _bacc.Bacc()
        build(nc, G)
        _CACHE["nc"] = nc
    return _CACHE["nc"]


def kernel(x, pos, tq, batch, w1a, b1a, w1b, b1b, w2a, b2a, w2b, b2b,
           wl1, bl1, wl2, bl2, wm1, bm1, wm2, bm2):
    """Full-input entry: shards graphs over 8 NeuronCores, returns [128, 3]."""
    from concourse.bass_utils import run_bass_kernel_spmd
    inputs = dict(x=np.asarray(x), pos=np.asarray(pos), tq=np.asarray(tq),
                  w1a=np.asarray(w1a), b1a=np.asarray(b1a),
                  w1b=np.asarray(w1b), b1b=np.asarray(b1b),
                  w2a=np.asarray(w2a), b2a=np.asarray(b2a),
                  w2b=np.asarray(w2b), b2b=np.asarray(b2b),
                  wl1=np.asarray(wl1), bl1=np.asarray(bl1),
                  wl2=np.asarray(wl2), bl2=np.asarray(bl2),
                  wm1=np.asarray(wm1), bm1=np.asarray(bm1),
                  wm2=np.asarray(wm2), bm2=np.asarray(bm2))
    NCORES = 8
    B_all = inputs["x"].shape[0] // N
    G = B_all // NCORES
    nc = _get_program(G)
    in_maps = [host_prep(inputs, G, c) for c in range(NCORES)]
    res = run_bass_kernel_spmd(nc, in_maps, core_ids=list(range(NCORES)))
    out = np.concatenate([res.results[c]["o"].T for c in range(NCORES)], axis=0)
    return out.astype(np.float32)



# revision 21
# speedup vs baseline: 1.5443x; 1.5187x over previous
"""DGCNN (nn_DGCNN_type1) Trainium2 Bass kernel — self-contained.

Strategy: data-parallel over the 128 graphs, 16 per NeuronCore across 8 cores.
Per graph: f16 kNN-score matmul (s = 2*f@f.T - |f_j|^2, row-constant dropped)
+ DVE max8/max_index/match_replace top-16 in f16; f16 edge-MLP via SBUF-source
dma_gather (token n at partition n%128, rank n//128, 256B each) with fp32 PSUM
accumulation; k-max folded through the monotone LeakyReLU as a PSUM-side
reduce; lin1 + global-max-pool + MLP head on-chip.

The graph loop is software-pipelined with a 2-graph lookahead on conv1's
knn+gather so the single SWDGE queue (the serial resource: ~8192 descriptors
per gather, one Pool engine) streams gathers back-to-back while PE/DVE/Act
compute runs underneath.
"""

import numpy as np
import concourse.bacc as bacc
import concourse.mybir as mybir
from concourse.tile import TileContext
from concourse.masks import make_identity

F32, F16, I16, U16 = (mybir.dt.float32, mybir.dt.float16, mybir.dt.int16,
                      mybir.dt.uint16)
AF = mybir.ActivationFunctionType
ALU = mybir.AluOpType
AX = mybir.AxisListType

N = 512
K = 16
SWDGE_QUEUES = 1     # >1 corrupts concurrent gathers on this deployment
GATHER_SPLIT = 1
LOOKAHEAD = 2        # conv1 knn+gather scheduled this many graphs ahead
# timing-bisect knobs (default = full kernel)
GATHER_NIDX = None   # override num_idxs per gather (timing only; breaks data)
SKIP_TOPK = False
NCHUNK = N // 128  # 4 row-chunks for the NxN score matrix


def host_prep(inputs, G, core):
    """Build the per-core in_map (numpy only: layout/dtype prep, no model math)."""
    f16 = np.float16
    x, pos, tq = inputs["x"], inputs["pos"], inputs["tq"]
    B_all = x.shape[0] // N
    xx = np.concatenate([tq, x, pos], axis=1).reshape(B_all, N, 5).astype(np.float32)
    sl = slice(core * G, (core + 1) * G)
    xxc = xx[sl]                                   # [G, 512, 5]
    # SBUF-gather layout: token n at partition n%128, rank n//128 (256B each);
    # partition-major so the one-shot preload DMA is contiguous
    xtab1 = np.zeros((128, G, NCHUNK, 128), f16)
    xtab1[:, :, :, 0:5] = xxc.reshape(G, NCHUNK, 128, 5).transpose(2, 0, 1, 3).astype(f16)

    w1a, w1b = inputs["w1a"], inputs["w1b"]
    w2a, w2b = inputs["w2a"], inputs["w2b"]
    w1botp = np.zeros((128, 64), f16)
    w1botp[0:5] = w1a[5:10].astype(f16)
    w2botp = np.zeros((128, 128), f16)
    w2botp[0:64] = w2a[64:128].astype(f16)
    wl1 = inputs["wl1"]

    return {
        "feat5h": np.ascontiguousarray(xxc.transpose(0, 2, 1)).astype(f16),
        "xtab1": xtab1,
        "w1mod": np.ascontiguousarray((w1a[0:5] - w1a[5:10]).astype(f16)),
        "w1botp": w1botp,
        "w1b": np.ascontiguousarray(w1b.astype(f16)),
        "b1a": inputs["b1a"].reshape(64, 1).astype(np.float32),
        "b1b2": np.tile(inputs["b1b"], 2).reshape(128, 1).astype(np.float32),
        "w2mod": np.ascontiguousarray((w2a[0:64] - w2a[64:128]).astype(f16)),
        "w2botp": w2botp,
        "w2b": np.ascontiguousarray(w2b.astype(f16)),
        "b2a": inputs["b2a"].reshape(128, 1).astype(np.float32),
        "b2b2": np.tile(inputs["b2b"], 2).reshape(128, 1).astype(np.float32),
        "wl1xx": np.ascontiguousarray(wl1[0:5].astype(f16)),
        "wl1x1": np.ascontiguousarray(wl1[5:69].astype(f16)),
        "wl1x2": np.ascontiguousarray(wl1[69:133].astype(f16)),
        "bl1c": np.ascontiguousarray(inputs["bl1"].reshape(4, 128).T.astype(np.float32)),
        "wl2": np.ascontiguousarray(inputs["wl2"].astype(f16).reshape(4, 128, 256).transpose(1, 0, 2)),
        "bl2c": np.ascontiguousarray(inputs["bl2"].reshape(2, 128).T.astype(np.float32)),
        "wm1": np.ascontiguousarray(inputs["wm1"].astype(f16).reshape(2, 128, 128).transpose(1, 0, 2)),
        "bm1": inputs["bm1"].reshape(128, 1).astype(np.float32),
        "wm2": np.ascontiguousarray(inputs["wm2"].astype(f16)),
        "bm2": inputs["bm2"].reshape(3, 1).astype(np.float32),
    }


def declare_io(nc, G):
    t = {}
    def inp(name, shape, dt):
        t[name] = nc.dram_tensor(name, shape, dt, kind="ExternalInput")
    inp("feat5h", [G, 5, N], F16)
    inp("xtab1", [128, G, NCHUNK, 128], F16)
    inp("w1mod", [5, 64], F16); inp("w1botp", [128, 64], F16)
    inp("w1b", [64, 64], F16); inp("b1a", [64, 1], F32); inp("b1b2", [128, 1], F32)
    inp("w2mod", [64, 128], F16); inp("w2botp", [128, 128], F16)
    inp("w2b", [128, 64], F16); inp("b2a", [128, 1], F32); inp("b2b2", [128, 1], F32)
    inp("wl1xx", [5, N], F16); inp("wl1x1", [64, N], F16); inp("wl1x2", [64, N], F16)
    inp("bl1c", [128, 4], F32); inp("wl2", [128, 4, 256], F16); inp("bl2c", [128, 2], F32)
    inp("wm1", [128, 2, 128], F16); inp("bm1", [128, 1], F32)
    inp("wm2", [128, 3], F16); inp("bm2", [3, 1], F32)
    t["o"] = nc.dram_tensor("o", [3, G], F32, kind="ExternalOutput")
    return t


def build(nc, G, reps=1):
    t = declare_io(nc, G)
    with TileContext(nc) as tc:
        _build_body(nc, tc, t, G, reps)
    nc.compile()
    return t


def _build_body(nc, tc, t, G, reps=1):
    sbw = tc.alloc_tile_pool(name="sbw", bufs=1)          # persistent
    sb = tc.alloc_tile_pool(name="sb", bufs=2)            # rotating tiles
    sb3 = tc.alloc_tile_pool(name="sb3", bufs=3)          # deeper rotation
    ps_nd = tc.alloc_tile_pool(name="ps_nd", bufs=2, space="PSUM")   # 2 banks
    ps_a1 = tc.alloc_tile_pool(name="ps_a1", bufs=2, space="PSUM")   # 2 banks
    ps_a2 = tc.alloc_tile_pool(name="ps_a2", bufs=1, space="PSUM")   # 2 banks
    ps_sm = tc.alloc_tile_pool(name="ps_sm", bufs=1, space="PSUM")   # 1 bank

    # ---- persistent weight tiles ----
    w = {}
    for name in ["w1mod", "w1botp", "w1b", "w2mod", "w2botp", "w2b",
                 "wl1xx", "wl1x1", "wl1x2", "wl2", "wm1", "wm2"]:
        w[name] = sbw.tile(list(t[name].shape), F16, tag=name, name='w_'+name)
        nc.sync.dma_start(out=w[name][:], in_=t[name][:])
    for name in ["b1a", "b1b2", "b2a", "b2b2", "bl1c", "bl2c", "bm1", "bm2"]:
        w[name] = sbw.tile(list(t[name].shape), F32, tag=name, name='b_'+name)
        nc.sync.dma_start(out=w[name][:], in_=t[name][:])
    ident = sbw.tile([128, 128], F16, tag="ident")
    make_identity(nc, ident[:])
    ones = sbw.tile([64, 1], F16, tag="ones")
    nc.gpsimd.memset(ones[:], 1.0)
    onesrow = sbw.tile([1, N], F16, tag="onesrow")
    nc.gpsimd.memset(onesrow[:], 1.0)

    # persistent gather-index tiles (rows 16-127 must hold valid values);
    # slots 0-1 rotate for conv1, slots 2-3 for conv2
    idx_tiles = []
    for s in range(4):
        it = sbw.tile([128, N], I16, tag=f"idxs{s}", name=f"idxs{s}")
        nc.gpsimd.memset(it[:], 0)
        idx_tiles.append(it)

    # conv1 gather table for all G graphs, resident in SBUF (16 KiB/partition)
    xtab_sb = sbw.tile([128, G, NCHUNK, 128], F16, tag="xtab_sb")
    nc.sync.dma_start(out=xtab_sb[:], in_=t["xtab1"][:])

    Gt_lo = sbw.tile([128, G], F32, tag="gtlo")
    Gt_hi = sbw.tile([128, G], F32, tag="gthi")

    rep_ctx = tc.For_i(0, reps, 1) if reps > 1 else None
    if rep_ctx is not None:
        rep_ctx.__enter__()

    def knn_phase(d, fill_B, idxs):
        """Score matrix + top-16 -> wrapped idx tile (rows 0:32)."""
        dp = d + 1
        B = sb.tile([65, N], F16, tag="Btile")
        fill_B(B)                              # fills B[0:d, :] (f16)
        sc = sb.tile([65, N], F16, tag="sctile")
        nc.vector.tensor_scalar_mul(sc[0:d, :], B[0:d, :], 2.0)
        F2 = sb.tile([64, N], F16, tag="F2tile")
        nc.scalar.activation(F2[0:d, :], B[0:d, :], AF.Square)
        sqp = ps_sm.tile([1, N], F32, tag="sm")
        nc.tensor.matmul(out=sqp[:], lhsT=ones[0:d, :], rhs=F2[0:d, :],
                         start=True, stop=True)
        if d % 32 == 0:
            nc.gpsimd.memset(sc[d:d + 1, :], 1.0)
            nc.scalar.activation(B[d:d + 1, :], sqp[:], AF.Identity, scale=-1.0)
        else:
            nc.sync.dma_start(out=sc[d:d + 1, :], in_=onesrow[:])
            sqtmp = sb.tile([1, N], F16, tag="sqtmp")
            nc.scalar.activation(sqtmp[:], sqp[:], AF.Identity, scale=-1.0)
            nc.sync.dma_start(out=B[d:d + 1, :], in_=sqtmp[:])

        idxTp = ps_sm.tile([16, N], F16, tag="sm")
        for c in range(NCHUNK):
            nd_p = ps_nd.tile([128, N], F32, tag="ndp")
            nc.tensor.matmul(out=nd_p[:], lhsT=sc[0:dp, 128 * c:128 * (c + 1)],
                             rhs=B[0:dp, :], start=True, stop=True)
            nd = sb.tile([128, N], F16, tag="ndsb")
            nc.scalar.activation(nd[:], nd_p[:], AF.Copy)
            maxv = sb.tile([128, 16], F16, tag="maxv")
            maxi = sb.tile([128, 16], U16, tag="maxi")
            if not SKIP_TOPK:
                nc.vector.max(out=maxv[:, 0:8], in_=nd[:])
                nc.vector.max_index(out=maxi[:, 0:8], in_max=maxv[:, 0:8], in_values=nd[:])
                nc.vector.match_replace(out=nd[:], in_to_replace=maxv[:, 0:8],
                                        in_values=nd[:], imm_value=-60000.0)
                nc.vector.max(out=maxv[:, 8:16], in_=nd[:])
                nc.vector.max_index(out=maxi[:, 8:16], in_max=maxv[:, 8:16], in_values=nd[:])
            else:
                nc.vector.memset(maxi[:], 0)
            mif = sb.tile([128, 16], F16, tag="mif")
            nc.vector.tensor_copy(mif[:], maxi[:])
            nc.tensor.transpose(out=idxTp[:, 128 * c:128 * (c + 1)], in_=mif[:],
                                identity=ident[:])
        # rows 0-15: full wrapped idxT (read by the sim + rx cpu);
        # rows 16-31: copy for the tx cpu (reads partitions 16-31).
        nc.vector.tensor_copy(idxs[0:16, :], idxTp[:])
        nc.sync.dma_start(out=idxs[16:32, :], in_=idxs[0:16, :])

    def gather_phase(idxs, gtab_ap, tag):
        """xj gather from SBUF table -> feature-major [128, 8192] f16."""
        xjg = sb.tile([128, K * N], F16, tag=tag)
        ni = GATHER_NIDX or (K * N)
        per = ni // GATHER_SPLIT
        assert per % 128 == 0
        for q in range(GATHER_SPLIT):
            nc.gpsimd.dma_gather(out_ap=xjg[:, None, per * q:per * (q + 1)],
                                 in_ap=gtab_ap,
                                 idxs_ap=idxs[:, per * q // 16:
                                              per * q // 16 + max(per // 16, 32)],
                                 num_idxs=per, num_idxs_reg=per, elem_size=128,
                                 transpose=True, single_packet=False,
                                 sbuf_tokens_per_rank=128,
                                 sbuf_free_dim_per_rank=256,
                                 queue_num=q % SWDGE_QUEUES)
        if ni < K * N:
            nc.gpsimd.memset(xjg[:, ni:], 0)
        return xjg

    def mlp_phase(xjg, d, Fmid, wmod, wbotp, wsec, ba, bb2, featTh, kact_sink):
        """Edge MLP layer1+layer2+kmax. kact_sink(tt, kact): kact [128, 64]
        f32, rows 64h+f = feature f of node group h; tile tt covers nodes
        [128*tt + 64*h, +64)."""
        for tt in range(4):
            a2 = ps_a2.tile([128, 1024], F32, tag="a2")
            for q in range(4):
                h, r = q // 2, q % 2
                c = 4 * tt + 2 * h + r
                a1 = ps_a1.tile([128, N], F32, tag="a1")
                nc.tensor.matmul(out=a1[0:Fmid, :], lhsT=wmod[:],
                                 rhs=featTh[:, 32 * c:32 * (c + 1), None]
                                 .to_broadcast([d, 32, K]),
                                 start=True, stop=False)
                nc.tensor.matmul(out=a1[0:Fmid, :], lhsT=wbotp[:],
                                 rhs=xjg[:, N * c:N * (c + 1)],
                                 start=False, stop=True)
                h1 = sb3.tile([128, N], F16, tag="h1")
                nc.scalar.activation(h1[0:Fmid, :], a1[0:Fmid, :], AF.Prelu,
                                     bias=ba[:], alpha=0.01)
                nc.tensor.matmul(out=a2[64 * h:64 * h + 64, 512 * r:512 * (r + 1)],
                                 lhsT=wsec[:], rhs=h1[0:Fmid, :],
                                 start=True, stop=True)
            kmx = sb.tile([128, 64], F32, tag="kmx")
            nc.vector.tensor_reduce(out=kmx[:], in_=a2[:].rearrange(
                "p (n k) -> p n k", k=K), op=ALU.max, axis=AX.X)
            kact = sb.tile([128, 64], F32, tag="kact")
            nc.scalar.activation(kact[:], kmx[:], AF.Prelu, bias=bb2[:], alpha=0.01)
            kact_sink(tt, kact)

    # ---- pipelined per-graph schedule ----
    f5h_t, xjg1_t = {}, {}

    def sched_knn1(g):
        """conv1 knn + gather for graph g (runs LOOKAHEAD graphs early)."""
        tl = sb3.tile([5, N], F16, tag="f5h")
        nc.sync.dma_start(out=tl[:], in_=t["feat5h"][g])
        f5h_t[g] = tl
        def fillB(B, g=g):
            nc.sync.dma_start(out=B[0:5, :], in_=t["feat5h"][g])
        idxs = idx_tiles[g % 2]
        knn_phase(5, fillB, idxs)
        xjg1_t[g] = gather_phase(idxs, xtab_sb[:, g], "xjg1")

    for g in range(min(LOOKAHEAD, G)):
        sched_knn1(g)

    for g in range(G):
        # ===================== conv1 MLP =====================
        f5h = f5h_t.pop(g)
        x1f16 = sb.tile([64, N], F16, tag="x1f16")

        def sink1(tt, kact):
            for h in range(2):
                cols = slice(128 * tt + 64 * h, 128 * tt + 64 * h + 64)
                nc.scalar.activation(x1f16[:, cols], kact[64 * h:64 * h + 64, :],
                                     AF.Copy)

        mlp_phase(xjg1_t.pop(g), 5, 64, w["w1mod"], w["w1botp"], w["w1b"],
                  w["b1a"], w["b1b2"], f5h, sink1)

        # x1 node-major staging -> SBUF table (cols 0:64 = x1, 64:128 = dup)
        x1nmp = ps_sm.tile([128, 256], F16, tag="sm")
        for c in range(NCHUNK):
            nc.tensor.transpose(out=x1nmp[:, 64 * c:64 * (c + 1)],
                                in_=x1f16[:, 128 * c:128 * (c + 1)],
                                identity=ident[0:64, 0:64])
        x1nm = sb.tile([128, 4, 128], F16, tag="x1nm")
        nc.scalar.activation(x1nm[:, :, 0:64],
                             x1nmp[:].rearrange("p (a b) -> p a b", b=64), AF.Copy)
        nc.scalar.activation(x1nm[:, :, 64:128],
                             x1nmp[:].rearrange("p (a b) -> p a b", b=64), AF.Copy)

        # ===================== conv2 knn + gather =====================
        def fillB2(B):
            nc.vector.tensor_copy(B[0:64, :], x1f16[:])
        idxs2 = idx_tiles[2 + g % 2]
        knn_phase(64, fillB2, idxs2)
        xjg2 = gather_phase(idxs2, x1nm[:], "xjg2")

        # conv1 knn + gather for graph g+LOOKAHEAD rides behind gather2(g)
        if g + LOOKAHEAD < G:
            sched_knn1(g + LOOKAHEAD)

        # ===================== conv2 MLP =====================
        x2f16 = sb.tile([64, N], F16, tag="x2f16")

        def sink2(tt, kact):
            for h in range(2):
                cols = slice(128 * tt + 64 * h, 128 * tt + 64 * h + 64)
                nc.scalar.activation(x2f16[:, cols], kact[64 * h:64 * h + 64, :],
                                     AF.Copy)

        mlp_phase(xjg2, 64, 128, w["w2mod"], w["w2botp"], w["w2b"],
                  w["b2a"], w["b2b2"], x1f16, sink2)

        # ===================== lin1 + pool =====================
        h2p = ps_a2.tile([128, 1024], F32, tag="a2")
        for c in range(NCHUNK):
            hp = ps_a1.tile([128, N], F32, tag="a1")
            nc.tensor.matmul(out=hp[:], lhsT=w["wl1xx"][:, 128 * c:128 * (c + 1)],
                             rhs=f5h[:], start=True, stop=False)
            nc.tensor.matmul(out=hp[:], lhsT=w["wl1x1"][:, 128 * c:128 * (c + 1)],
                             rhs=x1f16[:], start=False, stop=False)
            nc.tensor.matmul(out=hp[:], lhsT=w["wl1x2"][:, 128 * c:128 * (c + 1)],
                             rhs=x2f16[:], start=False, stop=True)
            hsb = sb3.tile([128, N], F16, tag="h1")
            nc.scalar.activation(hsb[:], hp[:], AF.Prelu,
                                 bias=w["bl1c"][:, c:c + 1], alpha=0.01)
            for fo in range(2):
                nc.tensor.matmul(out=h2p[:, N * fo:N * (fo + 1)],
                                 lhsT=w["wl2"][:, c, 128 * fo:128 * (fo + 1)],
                                 rhs=hsb[:], start=(c == 0), stop=(c == NCHUNK - 1))
        for fo, gt in enumerate((Gt_lo, Gt_hi)):
            nc.vector.tensor_reduce(out=gt[:, g:g + 1], in_=h2p[:, N * fo:N * (fo + 1)],
                                    op=ALU.max, axis=AX.X)

    # ===================== head =====================
    t1p = ps_sm.tile([128, G], F32, tag="sm")
    for fo in range(2):
        gt = Gt_lo if fo == 0 else Gt_hi
        ga = sb.tile([128, G], F16, tag="ga")
        nc.scalar.activation(ga[:], gt[:], AF.Prelu, bias=w["bl2c"][:, fo:fo + 1],
                             alpha=0.01)
        nc.tensor.matmul(out=t1p[:], lhsT=w["wm1"][:, fo, :],
                         rhs=ga[:], start=(fo == 0), stop=(fo == 1))
    t1 = sb.tile([128, G], F16, tag="t1")
    nc.scalar.activation(t1[:], t1p[:], AF.Prelu, bias=w["bm1"][:], alpha=0.01)
    outp = ps_sm.tile([3, G], F32, tag="sm")
    nc.tensor.matmul(out=outp[:], lhsT=w["wm2"][:], rhs=t1[:], start=True, stop=True)
    outsb = sb.tile([3, G], F32, tag="outsb")
    nc.scalar.activation(outsb[:], outp[:], AF.Identity, bias=w["bm2"][:])
    nc.sync.dma_start(out=t["o"][:], in_=outsb[:])

    if rep_ctx is not None:
        rep_ctx.__exit__(None, None, None)

    for pool in (ps_sm, ps_a2, ps_a1, ps_nd, sb3, sb, sbw):
        pool.release()


# ======================= harness entry point =======================
_CACHE = {}


def _get_program(G):
    if "nc" not in _CACHE:
        import concourse.bacc as _bacc
        nc = _bacc.Bacc(num_swdge_queues=SWDGE_QUEUES)
        build(nc, G)
        _CACHE["nc"] = nc
    return _CACHE["nc"]


def kernel(x, pos, tq, batch, w1a, b1a, w1b, b1b, w2a, b2a, w2b, b2b,
           wl1, bl1, wl2, bl2, wm1, bm1, wm2, bm2):
    """Full-input entry: shards graphs over 8 NeuronCores, returns [128, 3]."""
    from concourse.bass_utils import run_bass_kernel_spmd
    inputs = dict(x=np.asarray(x), pos=np.asarray(pos), tq=np.asarray(tq),
                  w1a=np.asarray(w1a), b1a=np.asarray(b1a),
                  w1b=np.asarray(w1b), b1b=np.asarray(b1b),
                  w2a=np.asarray(w2a), b2a=np.asarray(b2a),
                  w2b=np.asarray(w2b), b2b=np.asarray(b2b),
                  wl1=np.asarray(wl1), bl1=np.asarray(bl1),
                  wl2=np.asarray(wl2), bl2=np.asarray(bl2),
                  wm1=np.asarray(wm1), bm1=np.asarray(bm1),
                  wm2=np.asarray(wm2), bm2=np.asarray(bm2))
    NCORES = 8
    B_all = inputs["x"].shape[0] // N
    G = B_all // NCORES
    nc = _get_program(G)
    in_maps = [host_prep(inputs, G, c) for c in range(NCORES)]
    res = run_bass_kernel_spmd(nc, in_maps, core_ids=list(range(NCORES)))
    out = np.concatenate([res.results[c]["o"].T for c in range(NCORES)], axis=0)
    return out.astype(np.float32)
